# revision 7
# baseline (speedup 1.0000x reference)
"""C3D loss kernel for Trainium2 (8 NeuronCores, Bass/Tile).

Sharding: pure data parallel over B*2 = 8 shards (each image split into
top/bottom 176-row halves). Each core computes a partial sum of the loss
numerator; host combines and divides by the valid count.

Layout: partitions = 122 column blocks of 10 pixels (3+3 col halo -> 16
stored cols per block); free dims = (rows, 16). Every spatial shift (the
5x5 window and the normal central differences) is a free-dim offset, which
keeps all engine accesses at partition start 0 (a hardware requirement).

Out-of-image semantics (must match the reference's zero-pad + `vs` mask):
- normals are computed from zero-padded xyz (host-zero-padded slabs),
- afterwards the pred cloud's out-of-image halo is overwritten with a huge
  poison value so that exp(-200*d2) underflows to exactly 0 for any window
  offset that falls outside the image (DMA overwrites: per-core row strips
  + column poison), which reproduces `vs = 0` exactly.
"""
import sys

sys.path.insert(0, "/opt/trn_rl_repo")

import numpy as np
from contextlib import ExitStack

import bass_rust
import concourse.bass as bass
import concourse.tile as tile
from concourse import bacc, mybir
from concourse.bass_utils import run_bass_kernel_spmd

F32 = mybir.dt.float32
F16 = mybir.dt.float16
AF = mybir.ActivationFunctionType
ALU = mybir.AluOpType

# Problem constants
B, H, W = 4, 352, 1216
R = 2
ELL = 0.05
INV2ELL2 = float(np.float32(1.0 / (2.0 * ELL * ELL)))   # 200.0
EPS = 1e-8
N_CORES = 8

SH = H // 2          # shard rows per core = 176
NT = 2               # row tiles per core
TR = SH // NT        # output rows per tile = 88
RB = TR + 6          # stored rows per tile = 94
CB = 10              # cols per block
NB = 122             # blocks (122*10 = 1220 >= 1216)
BW = CB + 6          # stored cols per block = 16
SW = CB * (NB - 1) + BW   # slab width = 1226 (slab col j <-> image col j-3)
PZ = 1.0e6
LN14 = float(np.log(0.25))

_prog_cache = {}


def _win(ap3, r0, r1, c0, c1):
    return ap3[:, r0:r1, c0:c1]


def _build_program():
    nc = bacc.Bacc("TRN2", target_bir_lowering=False, debug=False,
                   num_devices=N_CORES)

    for v in (EPS, LN14):
        t = nc.alloc_sbuf_tensor(f"const-f32-{v}", [128, 1], F32)
        nc.gpsimd.memset(t.ap(), v)
        nc.const_aps.aps[(F32, v)] = t.ap()
    nc.all_engine_barrier()

    dp_d = nc.dram_tensor("dp", [SH + 6, SW], F32, kind="ExternalInput").ap()
    dg_d = nc.dram_tensor("dg", [SH + 6, SW], F32, kind="ExternalInput").ap()
    xy1_d = nc.dram_tensor("xy1", [3, SH + 6, SW], F32, kind="ExternalInput").ap()
    mk_d = nc.dram_tensor("mk", [SH, SW], F32, kind="ExternalInput").ap()
    # per-core strips: [top/bottom, channel, block, 3 rows, 16 cols]
    st_d = nc.dram_tensor("strip", [2, 3, NB, 3, BW], F32,
                          kind="ExternalInput").ap()
    pz_d = nc.dram_tensor("pzc", [RB, 2], F32, kind="ExternalInput").ap()
    id_d = nc.dram_tensor("idm", [NB, NB], F16, kind="ExternalInput").ap()
    out_d = nc.dram_tensor("out", [128, NT], F32, kind="ExternalOutput").ap()

    def slab_view(dram_ap, row0, nrows, extra_off=0):
        """[NB, nrows, BW] overlapping col-block window into a [*, SW] slab."""
        v = dram_ap.copy()
        v.ap = bass_rust.VecI64Pair([[CB, NB], [SW, nrows], [1, BW]])
        v.offset = v.offset + row0 * SW + extra_off
        return v

    with tile.TileContext(nc) as tc, ExitStack() as ctx:
        pool = ctx.enter_context(tc.tile_pool(name="p", bufs=1))
        psum = ctx.enter_context(tc.tile_pool(name="ps", bufs=1, space="PSUM"))
        idt = pool.tile([NB, NB], F16, name="idt")
        nc.sync.dma_start(out=idt[:], in_=id_d[:])

        for t in range(NT):
            r0 = t * TR

            # ---- load inputs ----
            dpt = pool.tile([NB, RB, BW], F32, name="dpt")
            nc.sync.dma_start(out=dpt[:], in_=slab_view(dp_d, r0, RB))
            dgt = pool.tile([NB, RB, BW], F32, name="dgt")
            nc.sync.dma_start(out=dgt[:], in_=slab_view(dg_d, r0, RB))
            xy1t = [pool.tile([NB, RB, BW], F32, name=f"xy1t{c}") for c in range(3)]
            for c in range(3):
                nc.sync.dma_start(out=xy1t[c][:],
                                  in_=slab_view(xy1_d[c], r0, RB))
            mkt = pool.tile([NB, TR, CB], F32, name="mkt")
            mv = mk_d.copy()
            mv.ap = bass_rust.VecI64Pair([[CB, NB], [SW, TR], [1, CB]])
            mv.offset = mv.offset + r0 * SW + 3
            nc.sync.dma_start(out=mkt[:], in_=mv)

            # ---- xyz = xy1 * depth ----
            xp = [pool.tile([NB, RB, BW], F32, name=f"xp{c}") for c in range(3)]
            xg = [pool.tile([NB, RB, BW], F32, name=f"xg{c}") for c in range(3)]
            for c in range(3):
                nc.vector.tensor_mul(xp[c][:], xy1t[c][:], dpt[:])
                nc.vector.tensor_mul(xg[c][:], xy1t[c][:], dgt[:])

            # ---- normals (valid on rows [1:93), cols [1:15)) ----
            def w3(x, dr, dc):
                return x[:, 1 + dr:93 + dr, 1 + dc:15 + dc]

            nrm = {}
            for key, xc in (("p", xp), ("g", xg)):
                gx = [pool.tile([NB, RB, BW], F32, name=f"gx{c}") for c in range(3)]
                gy = [pool.tile([NB, RB, BW], F32, name=f"gy{c}") for c in range(3)]
                for c in range(3):
                    nc.vector.tensor_sub(w3(gx[c], 0, 0), w3(xc[c], 0, 1),
                                         w3(xc[c], 0, -1))
                    nc.vector.tensor_sub(w3(gy[c], 0, 0), w3(xc[c], 1, 0),
                                         w3(xc[c], -1, 0))
                cr = [pool.tile([NB, RB, BW], F32, name=f"cr{c}") for c in range(3)]
                tA = pool.tile([NB, RB, BW], F32, name="tA")
                eng = nc.vector if key == "p" else nc.gpsimd
                for c in range(3):
                    a, b = (c + 1) % 3, (c + 2) % 3
                    nc.vector.tensor_mul(w3(cr[c], 0, 0), w3(gx[a], 0, 0),
                                         w3(gy[b], 0, 0))
                    eng.tensor_mul(w3(tA, 0, 0), w3(gx[b], 0, 0),
                                   w3(gy[a], 0, 0))
                    eng.tensor_sub(w3(cr[c], 0, 0), w3(cr[c], 0, 0),
                                   w3(tA, 0, 0))
                q = pool.tile([NB, RB, BW], F32, name="q")
                sqt = pool.tile([NB, RB, BW], F32, name="sqt")
                nc.scalar.activation(w3(q, 0, 0), w3(cr[0], 0, 0), AF.Square)
                nc.scalar.activation(w3(sqt, 0, 0), w3(cr[1], 0, 0), AF.Square)
                nc.gpsimd.tensor_add(w3(q, 0, 0), w3(q, 0, 0), w3(sqt, 0, 0))
                nc.scalar.activation(w3(sqt, 0, 0), w3(cr[2], 0, 0), AF.Square)
                nc.gpsimd.tensor_add(w3(q, 0, 0), w3(q, 0, 0), w3(sqt, 0, 0))
                # w = 0.25/(0.25*sqrt(q) + EPS), matching n/(|n|+eps)
                sw = pool.tile([NB, RB, BW], F32, name="sw")
                nc.scalar.activation(w3(sw, 0, 0), w3(q, 0, 0), AF.Sqrt,
                                     scale=0.0625)
                nc.scalar.activation(w3(sw, 0, 0), w3(sw, 0, 0), AF.Ln, bias=EPS)
                nc.scalar.activation(w3(sw, 0, 0), w3(sw, 0, 0), AF.Exp,
                                     scale=-1.0, bias=LN14)
                nt_ = [pool.tile([NB, RB, BW], F16, name=f"n{key}{c}")
                       for c in range(3)]
                for c in range(3):
                    nc.vector.tensor_mul(w3(nt_[c], 0, 0), w3(cr[c], 0, 0),
                                         w3(sw, 0, 0))
                nrm[key] = nt_
            npn, ngn = nrm["p"], nrm["g"]

            # ---- poison xp: row strips (per-core content) + OOB columns ----
            for c in range(3):
                if t == 0:
                    nc.sync.dma_start(out=xp[c][:, 0:3, :], in_=st_d[0, c])
                if t == NT - 1:
                    nc.sync.dma_start(out=xp[c][:, RB - 3:RB, :], in_=st_d[1, c])
                nc.sync.dma_start(out=xp[c][0:1, :, 1:3],
                                  in_=pz_d.unsqueeze(0))
                nc.sync.dma_start(out=xp[c][NB - 1:NB, :, 9:11],
                                  in_=pz_d.unsqueeze(0))

            # ---- accumulators (PSUM; one bank each, 440 f32 < 512) ----
            accA = psum.tile([NB, TR // 2, CB], F32, name="accA")
            accB = psum.tile([NB, TR // 2, CB], F32, name="accB")

            # ---- 5x5 window ----
            d2 = pool.tile([NB, TR, CB], F32, name="d2")
            sq2 = pool.tile([NB, TR, CB], F32, name="sq2")
            kgt = pool.tile([NB, TR, CB], F16, name="kgt")
            stt = pool.tile([NB, TR, CB], F16, name="stt")
            prt = pool.tile([NB, TR, CB], F16, name="prt")
            trm = pool.tile([NB, TR, CB], F16, name="trm")

            def sh(x, dy, dx):
                return x[:, 3 + dy:3 + TR + dy, 3 + dx:3 + CB + dx]

            noff = (2 * R + 1) ** 2
            for oi, (dy, dx) in enumerate(
                    [(dy, dx) for dy in range(-R, R + 1)
                     for dx in range(-R, R + 1)]):
                for c in range(3):
                    dst = d2 if c == 0 else sq2
                    nc.vector.tensor_sub(dst[:], sh(xp[c], dy, dx),
                                         sh(xg[c], 0, 0))
                    nc.scalar.activation(dst[:], dst[:], AF.Square)
                    if c == 1:
                        nc.vector.tensor_add(d2[:], d2[:], sq2[:])
                    elif c == 2:
                        nc.gpsimd.tensor_add(d2[:], d2[:], sq2[:])
                nc.scalar.activation(kgt[:], d2[:], AF.Exp, scale=-INV2ELL2)
                for c in range(3):
                    dst = stt if c == 0 else prt
                    nc.vector.tensor_mul(dst[:], sh(npn[c], dy, dx),
                                         sh(ngn[c], 0, 0))
                    if c > 0:
                        nc.vector.tensor_add(stt[:], stt[:], prt[:])
                nc.scalar.activation(stt[:], stt[:], AF.Abs)
                nc.vector.tensor_scalar(stt[:], stt[:], 1.9, 0.1,
                                        ALU.mult, ALU.add)
                nc.vector.tensor_mul(trm[:], stt[:], kgt[:])
                hh = TR // 2
                for ch, accX in enumerate((accA, accB)):
                    nc.tensor.matmul(
                        accX[:], idt[:],
                        trm[:, ch * hh:(ch + 1) * hh, :],
                        start=(oi == 0), stop=(oi == noff - 1))

            # ---- masked reduction ----
            pv = pool.tile([NB, TR, CB], F32, name="pv")
            hh = TR // 2
            nc.vector.tensor_mul(pv[:, 0:hh, :], accA[:], mkt[:, 0:hh, :])
            nc.vector.tensor_mul(pv[:, hh:TR, :], accB[:], mkt[:, hh:TR, :])
            red = pool.tile([NB, 1], F32, name="red")
            nc.vector.tensor_reduce(red[:], pv[:], mybir.AxisListType.XY,
                                    ALU.add)
            nc.sync.dma_start(out=out_d[0:NB, t:t + 1], in_=red[:])

    nc.compile()
    return nc


def _slab(img, lo_extra=3):
    """[182, SW] slab from a [H?, W] image block, zero-padded."""
    rows = img.shape[0]
    s = np.zeros((rows, SW), dtype=np.float32)
    s[:, 3:3 + W] = img
    return s


def _strips(xy1_b, dp_b, r0_img):
    """Window-phase xp values for slab rows [0:3) and [179:182)."""
    out = np.zeros((2, 3, NB, 3, BW), dtype=np.float32)
    for side, base in ((0, r0_img - 3), (1, r0_img + SH)):
        # image rows base..base+2
        vals = np.full((3, 3, SW), PZ, dtype=np.float32)
        for i in range(3):
            y = base + i
            if 0 <= y < H:
                row = np.full((3, SW), PZ, dtype=np.float32)
                row[:, 3:3 + W] = xy1_b[:, y, :] * dp_b[y, :]
                # image-col OOB positions stay PZ; cols -3 and >=1218 unread
                vals[:, i, :] = row
            # else: whole row stays PZ (out of image)
        for p in range(NB):
            out[side, :, p, :, :] = vals[:, :, CB * p:CB * p + BW]
    return out


def kernel(depth_pred, depth_gt, xy1_grid, K, mask):
    if "nc" not in _prog_cache:
        _prog_cache["nc"] = _build_program()
    nc = _prog_cache["nc"]

    dp = np.asarray(depth_pred, dtype=np.float32).reshape(B, H, W)
    dg = np.asarray(depth_gt, dtype=np.float32).reshape(B, H, W)
    xy1 = np.asarray(xy1_grid, dtype=np.float32)
    mk = np.asarray(mask).reshape(B, H, W)

    pzc = np.full((RB, 2), PZ, dtype=np.float32)
    idm = np.eye(NB, dtype=np.float16)
    in_maps = []
    for core in range(N_CORES):
        b, half = core // 2, core % 2
        r0 = half * SH
        lo, hi = r0 - 3, r0 + SH + 3
        slo, shi = max(lo, 0), min(hi, H)
        dps = np.zeros((SH + 6, SW), dtype=np.float32)
        dgs = np.zeros((SH + 6, SW), dtype=np.float32)
        xys = np.zeros((3, SH + 6, SW), dtype=np.float32)
        dps[slo - lo:shi - lo, 3:3 + W] = dp[b, slo:shi]
        dgs[slo - lo:shi - lo, 3:3 + W] = dg[b, slo:shi]
        xys[:, slo - lo:shi - lo, 3:3 + W] = xy1[b, :, slo:shi]
        mks = np.zeros((SH, SW), dtype=np.float32)
        mks[:, 3:3 + W] = mk[b, r0:r0 + SH]
        in_maps.append({
            "dp": dps, "dg": dgs, "xy1": xys, "mk": mks,
            "strip": _strips(xy1[b], dp[b], r0),
            "pzc": pzc, "idm": idm,
        })

    res = run_bass_kernel_spmd(nc, in_maps, list(range(N_CORES)))
    total = 0.0
    for core in range(N_CORES):
        total += res.results[core]["out"][0:NB, :].astype(np.float64).sum()
    nval = float(mk.sum(dtype=np.float64))
    return np.float32(-total / (nval + EPS))
